# revision 18
# baseline (speedup 1.0000x reference)
"""Bass/Tile TRN2 kernel for nn_MultiHeadAttention (B=2, T=2048, C=1024, H=16, D=64).

Sharding (8 cores): core c -> batch b = c // 4, heads [4*(c%4) .. 4*(c%4)+3]
(tensor-parallel on heads x data-parallel on batch). Each core computes its
4 heads' attention plus its slice of the output projection (rows of Wp for
its heads), producing a partial [T, C]; the host sums the 4 partials per
batch (the "all-reduce" is done host-side since the full output is gathered
host-side anyway).

v3 design (all matmul operands bf16; PSUM accumulate fp32):
  - Host pre-casts x.T and all weights to bf16 AND pre-arranges weights in
    the exact SBUF layout, so every input DMA is contiguous (fast).
  - Projections are emitted JUST-IN-TIME as PE filler inside the (otherwise
    ACT-bound) attention loops: only Q/K chunk 0 of pair 0 and V tiles 0-3
    run up front.  This keeps the PE densely busy for the whole kernel so
    the HAM clock gate stays at 2.4 GHz (a sparse PE re-throttles to 1.2).
  - Attention per qc chunk of 512 queries, PAIRS SEQUENTIAL: S.T tile =
    K.T.T @ Q.T restricted to unmasked columns, exp on ACT -> bf16 SBUF,
    causal staircase applied in-place by gpsimd affine_select on diagonal
    tiles, PV with fused ones-column rowsum (M=65).  PSUM: s_ps ring
    2x[128,1024] + ot_ps 2x[65,512] + filler ring 1x[128,1024] = 8 banks.
  - Normalization decoupled from PE: per head, drain ot_ps -> SBUF bf16
    then reciprocal (order chosen so the psum ring frees ASAP), K=1
    selector matmuls broadcast the recips, bf16 tensor_mul normalizes.
  - Output projection (lhsT = O.T [128,128t], rhs = Wp) threaded as PE
    filler; the final qc's out-proj runs on the freed "A" psum ring so its
    stage copies overlap.
"""

import numpy as np

B, T, C = 2, 2048, 1024
H = 16
D = C // H  # 64
N_CORES = 8
PAIRS = 2  # head-pairs per core
KC = C // 128  # 8 contraction chunks
TT = T // 128  # 16 T tiles
QC = T // 512  # 4 Tq chunks

_CACHE = {}


DEBUG = False


def _build():
    import concourse.mybir as mybir
    import concourse.tile as tile
    from concourse import bacc

    f32 = mybir.dt.float32
    bf16 = mybir.dt.bfloat16

    nc = bacc.Bacc("TRN2", target_bir_lowering=False, debug=False,
                   num_devices=N_CORES)

    # weights host-prearranged: wq/wk [PAIRS, 128, KC*128] (partition-major
    # SBUF layout), wv [128, KC*256], wp [PAIRS, 128, C]
    xT_d = nc.dram_tensor("xT", [C, T], bf16, kind="ExternalInput").ap()
    wq_d = nc.dram_tensor("wq", [PAIRS, 128, KC * 128], bf16,
                          kind="ExternalInput").ap()
    wk_d = nc.dram_tensor("wk", [PAIRS, 128, KC * 128], bf16,
                          kind="ExternalInput").ap()
    wv_d = nc.dram_tensor("wv", [128, KC * 256], bf16,
                          kind="ExternalInput").ap()
    wp_d = nc.dram_tensor("wp", [PAIRS, 128, C], bf16,
                          kind="ExternalInput").ap()
    hsel_d = nc.dram_tensor("hsel", [1, 128], bf16, kind="ExternalInput").ap()
    out_d = nc.dram_tensor("out", [T, C], f32, kind="ExternalOutput").ap()
    dbg = None
    if DEBUG:
        dbg = {
            "qt": nc.dram_tensor("dbg_qt", [PAIRS, 128, T], f32,
                                 kind="ExternalOutput").ap(),
            "kt": nc.dram_tensor("dbg_kt", [PAIRS, 128, T], f32,
                                 kind="ExternalOutput").ap(),
            "v": nc.dram_tensor("dbg_v", [128, TT, 4, 65], f32,
                                kind="ExternalOutput").ap(),
            "ot": nc.dram_tensor("dbg_ot", [PAIRS, 128, T], f32,
                                 kind="ExternalOutput").ap(),
        }

    with tile.TileContext(nc) as tc:
        _emit(nc, tc, tile, mybir, xT_d, wq_d, wk_d, wv_d, wp_d, hsel_d, out_d,
              dbg=dbg)

    nc.compile()
    return nc


def _emit(nc, tc, tile, mybir, xT_d, wq_d, wk_d, wv_d, wp_d, hsel_d, out_d,
          dbg=None):
    from contextlib import ExitStack

    f32 = mybir.dt.float32
    bf16 = mybir.dt.bfloat16
    Exp = mybir.ActivationFunctionType.Exp

    ctx = ExitStack()
    with ctx:
        # ---- pools (everything long-lived: projections now span the whole
        # kernel) ----
        qt_pool = ctx.enter_context(tc.tile_pool(name="qt", bufs=1))
        v_pool = ctx.enter_context(tc.tile_pool(name="v", bufs=1))
        ot_pool = ctx.enter_context(tc.tile_pool(name="ot", bufs=1))
        const_pool = ctx.enter_context(tc.tile_pool(name="const", bufs=1))
        wp_pool = ctx.enter_context(tc.tile_pool(name="wp", bufs=1))
        xt_pool = ctx.enter_context(tc.tile_pool(name="xt", bufs=1))
        w_pool = ctx.enter_context(tc.tile_pool(name="w", bufs=1))
        p_pool = ctx.enter_context(tc.tile_pool(name="p", bufs=4))
        otraw_pool = ctx.enter_context(tc.tile_pool(name="otraw", bufs=4))
        norm_pool = ctx.enter_context(tc.tile_pool(name="norm", bufs=2))
        stage_pool = ctx.enter_context(tc.tile_pool(name="stage", bufs=2))
        psum_pool = ctx.enter_context(tc.tile_pool(name="psum", bufs=1,
                                                   space="PSUM"))

        # ---- SBUF tiles ----
        qt_sb = [qt_pool.tile([128, T], bf16, tag=f"qt{p}", name=f"qt{p}")
                 for p in range(PAIRS)]
        kt_sb = [qt_pool.tile([128, T], bf16, tag=f"kt{p}", name=f"kt{p}")
                 for p in range(PAIRS)]
        # v_all[:, tk, hh, 0:64] = V tile for head hh; col 64 = ones (fused
        # rowsum row for the PV matmul)
        v_all = v_pool.tile([128, TT, 4, 65], bf16, tag="vall", name="v_all")
        nc.gpsimd.memset(v_all[:, :, :, 64:65], 1.0)
        ot_sb = [ot_pool.tile([128, T], bf16, tag=f"ot{p}", name=f"ot{p}")
                 for p in range(PAIRS)]
        wq_sb = [w_pool.tile([128, KC * 128], bf16, tag=f"wq{p}",
                             name=f"wq_{p}") for p in range(PAIRS)]
        wk_sb = [w_pool.tile([128, KC * 128], bf16, tag=f"wk{p}",
                             name=f"wk_{p}") for p in range(PAIRS)]
        wv_sb = w_pool.tile([128, KC * 256], bf16, tag="wv", name="wv_sb")
        wp_sb = [wp_pool.tile([128, C], bf16, tag=f"wp{p}", name=f"wpsb{p}")
                 for p in range(PAIRS)]
        xt = [xt_pool.tile([128, T], bf16, tag=f"xt{k}", name=f"xtsb{k}")
              for k in range(KC)]
        hsel_sb = const_pool.tile([1, 128], bf16, tag="hsel", name="hsel_sb")

        # ---- input DMAs, striped over two queues; everything contiguous.
        # Priority: wq0/wk0 + all of xT first (any projection chunk needs
        # every xT k-chunk), then the later-needed weights.
        nc.sync.dma_start(wq_sb[0][:], wq_d[0])
        nc.scalar.dma_start(wk_sb[0][:], wk_d[0])
        for k in range(0, KC, 2):
            nc.sync.dma_start(xt[k][:], xT_d[128 * k:128 * (k + 1), :])
            nc.scalar.dma_start(xt[k + 1][:],
                                xT_d[128 * (k + 1):128 * (k + 2), :])
        nc.sync.dma_start(wv_sb[:], wv_d[:])
        nc.scalar.dma_start(wq_sb[1][:], wq_d[1])
        nc.scalar.dma_start(wk_sb[1][:], wk_d[1])
        for p in range(PAIRS):
            nc.sync.dma_start(wp_sb[p][:], wp_d[p])
        nc.sync.dma_start(hsel_sb[:], hsel_d[:])

        # warm up the ACT exp table early (hides the 1.3us ACT_TABLE_LOAD)
        warm = const_pool.tile([1, 8], bf16, tag="warm", name="warm")
        nc.gpsimd.memset(warm[:], 0.0)
        nc.scalar.activation(warm[:], warm[:], Exp, scale=0.125)

        # ---- projection chunk emitters (each = 8 accumulating MMs + one
        # DVE drain; popped as PE filler between attention tiles) ----
        proj_done = set()

        def emit_q_chunk(p, c, tag="F"):
            key = ("q", p, c)
            if key in proj_done:
                return
            proj_done.add(key)
            ps = psum_pool.tile([128, 512], f32, tag=tag, name="qch_ps",
                                bufs=(1 if tag == "F" else 2))
            for k in range(KC):
                nc.tensor.matmul(
                    ps[:], wq_sb[p][:, 128 * k:128 * (k + 1)],
                    xt[k][:, 512 * c:512 * (c + 1)],
                    start=(k == 0), stop=(k == KC - 1))
            nc.vector.tensor_copy(qt_sb[p][:, 512 * c:512 * (c + 1)], ps[:])

        def emit_k_chunk(p, c, tag="F"):
            key = ("k", p, c)
            if key in proj_done:
                return
            proj_done.add(key)
            ps = psum_pool.tile([128, 512], f32, tag=tag, name="kch_ps",
                                bufs=(1 if tag == "F" else 2))
            for k in range(KC):
                nc.tensor.matmul(
                    ps[:], wk_sb[p][:, 128 * k:128 * (k + 1)],
                    xt[k][:, 512 * c:512 * (c + 1)],
                    start=(k == 0), stop=(k == KC - 1))
            nc.vector.tensor_copy(kt_sb[p][:, 512 * c:512 * (c + 1)], ps[:])

        def emit_v_tile(t_, tag="F"):
            key = ("v", t_)
            if key in proj_done:
                return
            proj_done.add(key)
            ps = psum_pool.tile([128, 256], f32, tag=tag, name="v_ps",
                                bufs=(1 if tag == "F" else 2))
            for k in range(KC):
                nc.tensor.matmul(
                    ps[:], xt[k][:, 128 * t_:128 * (t_ + 1)],
                    wv_sb[:, 256 * k:256 * (k + 1)],
                    start=(k == 0), stop=(k == KC - 1))
            nc.vector.tensor_copy(
                v_all[:, t_, :, 0:64],
                ps[:].rearrange("p (hh d) -> p hh d", hh=4))

        def emit_outproj(t_, tag="F"):
            o_ps = psum_pool.tile([128, 1024], f32, tag=tag, name="o_ps",
                                  bufs=(1 if tag == "F" else 2))
            for p2 in range(PAIRS):
                for c2 in range(2):
                    nc.tensor.matmul(
                        o_ps[:, 512 * c2:512 * (c2 + 1)],
                        ot_sb[p2][:, 128 * t_:128 * (t_ + 1)],
                        wp_sb[p2][:, 512 * c2:512 * (c2 + 1)],
                        start=(p2 == 0), stop=(p2 == PAIRS - 1),
                        skip_group_check=True)
            stage = stage_pool.tile([128, C], f32, tag="stage", name="stage")
            nc.vector.tensor_copy(stage[:], o_ps[:])
            nc.sync.dma_start(out_d[128 * t_:128 * (t_ + 1), :], stage[:])

        def emit_norm_tail(qc, p, rfb, otraw):
            # broadcast recip rows across partitions 0-63 via K=1 selector
            # matmuls (both heads side by side, base partition 0), then
            # normalize into ot_sb (off the PV critical path)
            bc_ps = psum_pool.tile([64, 1024], f32, tag="F", name="bc_ps",
                                   bufs=1)
            for h in range(2):
                nc.tensor.matmul(bc_ps[:, 512 * h:512 * (h + 1)],
                                 hsel_sb[:, 0:64], rfb[h][:],
                                 start=True, stop=True)
            bc_sb = norm_pool.tile([64, 1024], bf16, tag="bc", name="bc_sb")
            nc.vector.tensor_copy(bc_sb[:], bc_ps[:])
            for h in range(2):
                nc.vector.tensor_mul(
                    ot_sb[p][64 * h:64 * (h + 1), 512 * qc:512 * (qc + 1)],
                    otraw[h][0:64, :],
                    bc_sb[:, 512 * h:512 * (h + 1)])

        # need-ordered filler queue of projection chunks (what each
        # attention segment requires, minus the pre-emitted head start)
        def seg_needs(qc, p):
            needs = []
            if p == 0:
                needs += [("v", t_) for t_ in range(4 * qc, 4 * qc + 4)]
            needs += [("k", p, qc), ("q", p, qc)]
            return needs

        proj_queue = []
        for qc in range(QC):
            for p in range(PAIRS):
                proj_queue += seg_needs(qc, p)
        head_start = [("v", 0), ("v", 1), ("v", 2), ("v", 3),
                      ("k", 0, 0), ("q", 0, 0)]
        proj_queue = [k for k in proj_queue if k not in head_start]

        def emit_key(key, tag="F"):
            if key[0] == "q":
                emit_q_chunk(key[1], key[2], tag)
            elif key[0] == "k":
                emit_k_chunk(key[1], key[2], tag)
            else:
                emit_v_tile(key[1], tag)

        fillers = []  # deferred non-proj PE closures (bcast, out-proj)

        def pop_filler():
            # prefer projection chunks (they feed the attention critical
            # path); fall back to norm/out-proj closures
            while proj_queue and proj_queue[0] in proj_done:
                proj_queue.pop(0)
            if proj_queue:
                emit_key(proj_queue.pop(0))
            elif fillers:
                fillers.pop(0)()

        # ---- head start: the minimum projection work for segment (0,0),
        # on the double-buffered "A" ring (nothing competes yet) ----
        emit_v_tile(0, tag="A")
        emit_v_tile(1, tag="A")
        emit_k_chunk(0, 0, tag="A")
        emit_q_chunk(0, 0, tag="A")
        emit_v_tile(2, tag="A")
        emit_v_tile(3, tag="A")

        # ---- attention (qc-outer, pairs sequential) ----
        for qc in range(QC):
            n_tk = 4 * (qc + 1)
            for p in range(PAIRS):
                for key in seg_needs(qc, p):
                    emit_key(key)  # safety net: force any missing chunk
                ot_ps = [psum_pool.tile([65, 512], f32, tag="O", bufs=2,
                                        name=f"ot_ps{h}") for h in range(2)]
                for tk in range(n_tk):
                    j = tk - 4 * qc  # diagonal sub-position on diag tiles
                    lo = 128 * j if j > 0 else 0
                    s_ps = psum_pool.tile([128, 1024], f32, tag="A",
                                          name="s_ps", bufs=2)
                    for h in range(2):
                        nc.tensor.matmul(
                            s_ps[:, 512 * h + lo:512 * (h + 1)],
                            kt_sb[p][64 * h:64 * (h + 1),
                                     128 * tk:128 * (tk + 1)],
                            qt_sb[p][64 * h:64 * (h + 1),
                                     512 * qc + lo:512 * (qc + 1)],
                            start=True, stop=True,
                            tile_position=(64 * h, 0),
                        )
                    p_sb = p_pool.tile([128, 1024], bf16, tag="p_sb")
                    s3 = s_ps[:].rearrange("p (a q) -> p a q", a=2)
                    p3 = p_sb[:].rearrange("p (a q) -> p a q", a=2)
                    nc.scalar.activation(p3[:, :, lo:512], s3[:, :, lo:512],
                                         Exp, scale=0.125)
                    if j >= 0:
                        # causal staircase: zero entries with q < k in the
                        # 128-wide diagonal block, in place on gpsimd
                        nc.gpsimd.affine_select(
                            out=p3[:, :, lo:lo + 128],
                            in_=p3[:, :, lo:lo + 128],
                            compare_op=mybir.AluOpType.is_ge,
                            fill=0.0,
                            base=0,
                            pattern=[[0, 2], [1, 128]],
                            channel_multiplier=-1,
                        )
                    first, last = (tk == 0), (tk == n_tk - 1)
                    for h in range(2):
                        nc.tensor.matmul(
                            ot_ps[h][:, lo:512],
                            v_all[:, tk, 2 * p + h, :],
                            p_sb[:, 512 * h + lo:512 * (h + 1)],
                            start=first, stop=last,
                        )
                    pop_filler()

                # per head: drain then recip (this order frees the psum
                # ring slot as early as possible for the next segment)
                rfb = [norm_pool.tile([1, 512], bf16, tag=f"rfb{h}",
                                      name=f"rfb{h}") for h in range(2)]
                otraw = [otraw_pool.tile([65, 512], bf16, tag="otraw",
                                         name=f"otraw{h}") for h in range(2)]
                rfs = []
                for h in range(2):
                    rf = norm_pool.tile([65, 512], f32, tag="recipf",
                                        name="recipf", bufs=2)
                    nc.vector.tensor_copy(otraw[h][:], ot_ps[h][:])
                    nc.vector.reciprocal_approx_fast(rf[:], ot_ps[h][:])
                    rfs.append(rf)
                for h in range(2):
                    nc.vector.tensor_copy(rfb[h][:], rfs[h][64:65, :])
                fillers.append(
                    lambda qc=qc, p=p, r=rfb, o=otraw:
                    emit_norm_tail(qc, p, r, o))

            # after both pairs of qc: queue the output projection of qc
            for t_ in range(4 * qc, 4 * qc + 4):
                fillers.append(lambda t_=t_: emit_outproj(t_))

        # tail: remaining fillers (last qc's norm + out-proj) on the freed
        # double-buffered "A" ring so stage copies overlap the matmuls
        if fillers:
            fillers.pop(0)()  # norm tail of (QC-1, 1): tag F unused now
        for t_ in range(4 * (QC - 1), 4 * QC):
            emit_outproj(t_, tag="A")

        if dbg is not None:
            dstage_pool = ctx.enter_context(tc.tile_pool(name="dbgstage",
                                                         bufs=2))

            def dump(dst, src_ap, shape):
                st = dstage_pool.tile(shape, f32, tag="dstage", name="dstage")
                nc.vector.tensor_copy(st[:], src_ap)
                nc.sync.dma_start(dst, st[:])
            for p in range(PAIRS):
                dump(dbg["qt"][p], qt_sb[p][:], [128, T])
                dump(dbg["kt"][p], kt_sb[p][:], [128, T])
                dump(dbg["ot"][p], ot_sb[p][:], [128, T])
            dump(dbg["v"][:], v_all[:], [128, TT, 4, 65])


def _get_nc():
    if "nc" not in _CACHE:
        _CACHE["nc"] = _build()
    return _CACHE["nc"]


def make_in_maps(x, Wq, Wk, Wv, Wp):
    import ml_dtypes

    bf16 = ml_dtypes.bfloat16
    x = np.asarray(x, dtype=np.float32)
    Wq = np.asarray(Wq, dtype=np.float32).astype(bf16)
    Wk = np.asarray(Wk, dtype=np.float32).astype(bf16)
    Wv = np.asarray(Wv, dtype=np.float32).astype(bf16)
    Wp = np.asarray(Wp, dtype=np.float32).astype(bf16)

    hsel = np.zeros((1, 128), dtype=bf16)
    hsel[0, 0:64] = 1.0

    def sb_weight(wfull):
        # [C, M] -> SBUF layout [128, KC*M]: partition p holds, for each
        # k-chunk, row 128*k+p of the weight
        m = wfull.shape[1]
        return np.ascontiguousarray(
            wfull.reshape(KC, 128, m).transpose(1, 0, 2).reshape(128, -1))

    in_maps = []
    for c in range(N_CORES):
        b = c // 4
        h0 = 4 * (c % 4)  # first of the 4 heads on this core
        hs = list(range(h0, h0 + 4))
        xT = np.ascontiguousarray(x[b].T).astype(bf16)  # [C, T]
        wq = np.stack([sb_weight(np.concatenate(
            [Wq[hs[2 * p]], Wq[hs[2 * p + 1]]], axis=1)) for p in range(PAIRS)])
        wk = np.stack([sb_weight(np.concatenate(
            [Wk[hs[2 * p]], Wk[hs[2 * p + 1]]], axis=1)) for p in range(PAIRS)])
        wv = sb_weight(np.concatenate([Wv[h] for h in hs], axis=1))
        wp = np.stack([Wp[D * h0 + 128 * p:D * h0 + 128 * (p + 1), :]
                       for p in range(PAIRS)])
        in_maps.append({"xT": xT, "wq": wq, "wk": wk, "wv": wv, "wp": wp,
                        "hsel": hsel})
    return in_maps


def kernel(x, Wq, Wk, Wv, Wp):
    from concourse.bass_utils import run_bass_kernel_spmd

    in_maps = make_in_maps(x, Wq, Wk, Wv, Wp)
    nc = _get_nc()
    res = run_bass_kernel_spmd(nc, in_maps, list(range(N_CORES)))

    out = np.zeros((B, T, C), dtype=np.float32)
    for c in range(N_CORES):
        out[c // 4] += res.results[c]["out"]
    return out


# revision 32
# speedup vs baseline: 1.0158x; 1.0158x over previous
"""Bass/Tile TRN2 kernel for nn_MultiHeadAttention (B=2, T=2048, C=1024, H=16, D=64).

Sharding (8 cores): core c -> batch b = c // 4, heads [4*(c%4) .. 4*(c%4)+3]
(tensor-parallel on heads x data-parallel on batch). Each core computes its
4 heads' attention plus its slice of the output projection (rows of Wp for
its heads), producing a partial [T, C]; the host sums the 4 partials per
batch (the "all-reduce" is done host-side since the full output is gathered
host-side anyway).

v4 design (all matmul operands bf16; PSUM accumulate fp32):
  - Host pre-casts x.T and all weights to bf16 AND pre-arranges weights in
    the exact SBUF layout, so every input DMA is contiguous; xT is striped
    over three DMA queues (sync/scalar/vector).
  - k-paced head start: while xT streams in, the PE accumulates Q/K chunk 0
    of both pairs plus V tiles 0-3 chunk-by-chunk, consuming each xT
    k-chunk as it lands.
  - Remaining projections are emitted JUST-IN-TIME as PE filler inside the
    (otherwise ACT-bound) attention loops.  A densely busy PE keeps the HAM
    clock gate at 2.4 GHz (a sparse PE re-throttles to 1.2).
  - Attention per qc chunk of 512 queries, PAIRS SEQUENTIAL: S.T tile =
    K.T.T @ Q.T restricted to unmasked columns, exp on ACT -> bf16 SBUF,
    causal staircase applied in-place by gpsimd affine_select on diagonal
    tiles, PV with fused ones-column rowsum (M=65).  PSUM: s_ps ring
    2x[128,1024] + ot_ps ring 3x[65,512] + filler ring 1x[128,512] = 8
    banks.
  - Normalization decoupled from PE: per head, drain ot_ps -> SBUF bf16
    then reciprocal (order frees the psum ring ASAP); K=1 selector matmuls
    broadcast the recips; bf16 tensor_mul normalizes.  Norm tails are
    popped early (next segment, tile 2); out-projection half-tiles fill
    whatever PE slack remains, spread across the back half of the kernel.
"""

import numpy as np

B, T, C = 2, 2048, 1024
H = 16
D = C // H  # 64
N_CORES = 8
PAIRS = 2  # head-pairs per core
KC = C // 128  # 8 contraction chunks
TT = T // 128  # 16 T tiles
QC = T // 512  # 4 Tq chunks

_CACHE = {}


DEBUG = False


def _build():
    import concourse.mybir as mybir
    import concourse.tile as tile
    from concourse import bacc

    f32 = mybir.dt.float32
    bf16 = mybir.dt.bfloat16

    nc = bacc.Bacc("TRN2", target_bir_lowering=False, debug=False,
                   num_devices=N_CORES)

    # weights host-prearranged: wq/wk [PAIRS, 128, KC*128] (partition-major
    # SBUF layout), wv [128, KC*256], wp [PAIRS, 128, C]
    xT_d = nc.dram_tensor("xT", [C, T], bf16, kind="ExternalInput").ap()
    wq_d = nc.dram_tensor("wq", [PAIRS, 128, KC * 128], bf16,
                          kind="ExternalInput").ap()
    wk_d = nc.dram_tensor("wk", [PAIRS, 128, KC * 128], bf16,
                          kind="ExternalInput").ap()
    wv_d = nc.dram_tensor("wv", [128, KC * 256], bf16,
                          kind="ExternalInput").ap()
    wp_d = nc.dram_tensor("wp", [PAIRS, 128, C], bf16,
                          kind="ExternalInput").ap()
    hsel_d = nc.dram_tensor("hsel", [1, 128], bf16, kind="ExternalInput").ap()
    out_d = nc.dram_tensor("out", [T, C], f32, kind="ExternalOutput").ap()
    dbg = None
    if DEBUG:
        dbg = {
            "qt": nc.dram_tensor("dbg_qt", [PAIRS, 128, T], f32,
                                 kind="ExternalOutput").ap(),
            "kt": nc.dram_tensor("dbg_kt", [PAIRS, 128, T], f32,
                                 kind="ExternalOutput").ap(),
            "v": nc.dram_tensor("dbg_v", [128, TT, 4, 65], f32,
                                kind="ExternalOutput").ap(),
            "ot": nc.dram_tensor("dbg_ot", [PAIRS, 128, T], f32,
                                 kind="ExternalOutput").ap(),
            "wv": nc.dram_tensor("dbg_wv", [128, KC * 256], f32,
                                 kind="ExternalOutput").ap(),
        }

    with tile.TileContext(nc) as tc:
        _emit(nc, tc, tile, mybir, xT_d, wq_d, wk_d, wv_d, wp_d, hsel_d, out_d,
              dbg=dbg)

    nc.compile()
    return nc


def _emit(nc, tc, tile, mybir, xT_d, wq_d, wk_d, wv_d, wp_d, hsel_d, out_d,
          dbg=None):
    from contextlib import ExitStack

    f32 = mybir.dt.float32
    bf16 = mybir.dt.bfloat16
    Exp = mybir.ActivationFunctionType.Exp

    ctx = ExitStack()
    with ctx:
        # ---- pools (everything long-lived: projections span the whole
        # kernel) ----
        qt_pool = ctx.enter_context(tc.tile_pool(name="qt", bufs=1))
        v_pool = ctx.enter_context(tc.tile_pool(name="v", bufs=1))
        ot_pool = ctx.enter_context(tc.tile_pool(name="ot", bufs=1))
        const_pool = ctx.enter_context(tc.tile_pool(name="const", bufs=1))
        wp_pool = ctx.enter_context(tc.tile_pool(name="wp", bufs=1))
        xt_pool = ctx.enter_context(tc.tile_pool(name="xt", bufs=1))
        w_pool = ctx.enter_context(tc.tile_pool(name="w", bufs=1))
        p_pool = ctx.enter_context(tc.tile_pool(name="p", bufs=4))
        otraw_pool = ctx.enter_context(tc.tile_pool(name="otraw", bufs=4))
        norm_pool = ctx.enter_context(tc.tile_pool(name="norm", bufs=2))
        stage_pool = ctx.enter_context(tc.tile_pool(name="stage", bufs=3))
        psum_pool = ctx.enter_context(tc.tile_pool(name="psum", bufs=1,
                                                   space="PSUM"))

        # ---- SBUF tiles ----
        qt_sb = [qt_pool.tile([128, T], bf16, tag=f"qt{p}", name=f"qt{p}")
                 for p in range(PAIRS)]
        kt_sb = [qt_pool.tile([128, T], bf16, tag=f"kt{p}", name=f"kt{p}")
                 for p in range(PAIRS)]
        # v_all[:, tk, hh, 0:64] = V tile for head hh; col 64 = ones (fused
        # rowsum row for the PV matmul)
        v_all = v_pool.tile([128, TT, 4, 65], bf16, tag="vall", name="v_all")
        nc.gpsimd.memset(v_all[:, :, :, 64:65], 1.0)
        ot_sb = [ot_pool.tile([128, T], bf16, tag=f"ot{p}", name=f"ot{p}")
                 for p in range(PAIRS)]
        wq_sb = [w_pool.tile([128, KC * 128], bf16, tag=f"wq{p}",
                             name=f"wq_{p}") for p in range(PAIRS)]
        wk_sb = [w_pool.tile([128, KC * 128], bf16, tag=f"wk{p}",
                             name=f"wk_{p}") for p in range(PAIRS)]
        wv_sb = w_pool.tile([128, KC * 256], bf16, tag="wv", name="wv_sb")
        wp_sb = [wp_pool.tile([128, C], bf16, tag=f"wp{p}", name=f"wpsb{p}")
                 for p in range(PAIRS)]
        xt = [xt_pool.tile([128, T], bf16, tag=f"xt{k}", name=f"xtsb{k}")
              for k in range(KC)]
        hsel_sb = const_pool.tile([1, 128], bf16, tag="hsel", name="hsel_sb")

        # ---- input DMAs: the four Q/K weights + xT (in k order) race ahead
        # on the two HWDGE queues (the k-paced head start consumes each
        # xT chunk as it lands); later-needed weights go via gpsimd SWDGE.
        nc.sync.dma_start(wq_sb[0][:], wq_d[0])
        nc.scalar.dma_start(wk_sb[0][:], wk_d[0])
        nc.sync.dma_start(wq_sb[1][:], wq_d[1])
        nc.scalar.dma_start(wk_sb[1][:], wk_d[1])
        nc.scalar.dma_start(wv_sb[:], wv_d[:])
        qs = [nc.sync, nc.scalar]
        for k in range(KC):
            qs[k % 2].dma_start(xt[k][:], xT_d[128 * k:128 * (k + 1), :])
        nc.scalar.dma_start(wp_sb[0][:], wp_d[0])
        nc.sync.dma_start(wp_sb[1][:], wp_d[1])
        nc.scalar.dma_start(hsel_sb[:], hsel_d[:])

        # warm up the ACT exp table early (hides the 1.3us ACT_TABLE_LOAD)
        warm = const_pool.tile([1, 8], bf16, tag="warm", name="warm")
        nc.gpsimd.memset(warm[:], 0.0)
        nc.scalar.activation(warm[:], warm[:], Exp, scale=0.125)

        # ---- projection chunk emitters (each = 8 accumulating MMs + one
        # DVE drain; popped as PE filler between attention tiles) ----
        proj_done = set()

        def emit_q_chunk(p, c, tag="F"):
            key = ("q", p, c)
            if key in proj_done:
                return
            proj_done.add(key)
            ps = psum_pool.tile([128, 512], f32, tag=tag, name="qch_ps",
                                bufs={"F": 1, "A": 2, "O": 3}[tag])
            for k in range(KC):
                nc.tensor.matmul(
                    ps[:], wq_sb[p][:, 128 * k:128 * (k + 1)],
                    xt[k][:, 512 * c:512 * (c + 1)],
                    start=(k == 0), stop=(k == KC - 1))
            nc.vector.tensor_copy(qt_sb[p][:, 512 * c:512 * (c + 1)], ps[:])

        def emit_k_chunk(p, c, tag="F"):
            key = ("k", p, c)
            if key in proj_done:
                return
            proj_done.add(key)
            ps = psum_pool.tile([128, 512], f32, tag=tag, name="kch_ps",
                                bufs={"F": 1, "A": 2, "O": 3}[tag])
            for k in range(KC):
                nc.tensor.matmul(
                    ps[:], wk_sb[p][:, 128 * k:128 * (k + 1)],
                    xt[k][:, 512 * c:512 * (c + 1)],
                    start=(k == 0), stop=(k == KC - 1))
            nc.vector.tensor_copy(kt_sb[p][:, 512 * c:512 * (c + 1)], ps[:])

        def emit_v_tile(t_, tag="F"):
            key = ("v", t_)
            if key in proj_done:
                return
            proj_done.add(key)
            ps = psum_pool.tile([128, 256], f32, tag=tag, name="v_ps",
                                bufs={"F": 1, "A": 2, "O": 3}[tag])
            for k in range(KC):
                nc.tensor.matmul(
                    ps[:], xt[k][:, 128 * t_:128 * (t_ + 1)],
                    wv_sb[:, 256 * k:256 * (k + 1)],
                    start=(k == 0), stop=(k == KC - 1))
            nc.vector.tensor_copy(
                v_all[:, t_, :, 0:64],
                ps[:].rearrange("p (hh d) -> p hh d", hh=4))

        def emit_outproj_half(t_, c2, tag="F"):
            o_ps = psum_pool.tile([128, 512], f32, tag=tag, name="o_ps",
                                  bufs={"F": 1, "A": 2, "O": 3}[tag])
            for p2 in range(PAIRS):
                nc.tensor.matmul(
                    o_ps[:],
                    ot_sb[p2][:, 128 * t_:128 * (t_ + 1)],
                    wp_sb[p2][:, 512 * c2:512 * (c2 + 1)],
                    start=(p2 == 0), stop=(p2 == PAIRS - 1))
            stage = stage_pool.tile([128, 512], f32, tag="stage", name="stage")
            nc.vector.tensor_copy(stage[:], o_ps[:])
            nc.sync.dma_start(
                out_d[128 * t_:128 * (t_ + 1), 512 * c2:512 * (c2 + 1)],
                stage[:])

        def emit_norm_tail(qc, p, rfb, otraw):
            norm_emitted.add((qc, p))
            # broadcast recip rows across partitions 0-63 via K=1 selector
            # matmuls (base partition 0 so the bf16 tensor_mul has SBUF
            # inputs with matching bases), then normalize into ot_sb
            bc_sb = norm_pool.tile([64, 1024], bf16, tag="bc", name="bc_sb")
            for h in range(2):
                bc_ps = psum_pool.tile([64, 512], f32, tag="F", name="bc_ps",
                                       bufs=1)
                nc.tensor.matmul(bc_ps[:], hsel_sb[:, 0:64], rfb[h][:],
                                 start=True, stop=True)
                nc.vector.tensor_copy(bc_sb[:, 512 * h:512 * (h + 1)],
                                      bc_ps[:])
            for h in range(2):
                nc.vector.tensor_mul(
                    ot_sb[p][64 * h:64 * (h + 1), 512 * qc:512 * (qc + 1)],
                    otraw[h][0:64, :],
                    bc_sb[:, 512 * h:512 * (h + 1)])

        # need-ordered filler queue of projection chunks (what each
        # attention segment requires, minus the pre-emitted head start)
        def seg_needs(qc, p):
            needs = []
            if p == 0:
                needs += [("v", t_) for t_ in range(4 * qc, 4 * qc + 4)]
            needs += [("k", p, qc), ("q", p, qc)]
            return needs

        head_start = {("k", 0, 0), ("q", 0, 0), ("k", 0, 1), ("q", 0, 1),
                      ("k", 1, 0), ("q", 1, 0),
                      ("v", 0), ("v", 1), ("v", 2), ("v", 3)}
        proj_queue = []
        for qc in range(QC):
            for p in range(PAIRS):
                proj_queue += [k for k in seg_needs(qc, p)
                               if k not in head_start]

        def emit_key(key, tag="F"):
            if key[0] == "q":
                emit_q_chunk(key[1], key[2], tag)
            elif key[0] == "k":
                emit_k_chunk(key[1], key[2], tag)
            else:
                emit_v_tile(key[1], tag)

        norm_q = []  # norm-tail closures: popped at tile 2 of the next seg
        out_q = []   # out-proj half-tiles (t_, c2): fill remaining PE slack
        norm_emitted = set()  # (qc, p) pairs whose norm tail has been emitted

        def pop_filler(tk):
            if tk >= 2 and norm_q:
                norm_q.pop(0)()
                return
            while proj_queue and proj_queue[0] in proj_done:
                proj_queue.pop(0)
            if proj_queue:
                emit_key(proj_queue.pop(0))
                return
            # out-proj of qc is only valid once BOTH pairs' norm tails are
            # emitted (emission order, not just semaphores: an earlier read
            # would see stale SBUF)
            for i, (t_, c2) in enumerate(out_q):
                qc_t = t_ // 4
                if (qc_t, 0) in norm_emitted and (qc_t, 1) in norm_emitted:
                    out_q.pop(i)
                    emit_outproj_half(t_, c2)
                    return

        # ---- k-paced head start: consume each xT chunk as it lands,
        # accumulating Q/K chunk 0 of both pairs + Q/K chunk 1 of pair 0
        # (each accumulation group gets its own PSUM bank).
        # PSUM: 2 tag-A slots [128,1024] (2 banks each) + 2 tag-O slots.
        hs_a0 = psum_pool.tile([128, 1024], f32, tag="A", bufs=2,
                               name="hs_a0")
        hs_a1 = psum_pool.tile([128, 1024], f32, tag="A", bufs=2,
                               name="hs_a1")
        hs_q1 = psum_pool.tile([128, 512], f32, tag="O", bufs=3, name="hs_q1")
        hs_k1 = psum_pool.tile([128, 512], f32, tag="O", bufs=3, name="hs_k1")
        hs_mms = [
            (hs_a0[:, 0:512], wq_sb[0], 0),    # Q p0 c0
            (hs_a0[:, 512:1024], wk_sb[0], 0),  # K p0 c0
            (hs_a1[:, 0:512], wq_sb[0], 1),    # Q p0 c1
            (hs_a1[:, 512:1024], wk_sb[0], 1),  # K p0 c1
            (hs_q1[:], wq_sb[1], 0),           # Q p1 c0
            (hs_k1[:], wk_sb[1], 0),           # K p1 c0
        ]
        for k in range(KC):
            st, sp = (k == 0), (k == KC - 1)
            for (dst, w_, c) in hs_mms:
                nc.tensor.matmul(dst, w_[:, 128 * k:128 * (k + 1)],
                                 xt[k][:, 512 * c:512 * (c + 1)],
                                 start=st, stop=sp, skip_group_check=True)
        nc.vector.tensor_copy(qt_sb[0][:, 0:512], hs_a0[:, 0:512])
        nc.vector.tensor_copy(kt_sb[0][:, 0:512], hs_a0[:, 512:1024])
        nc.vector.tensor_copy(qt_sb[0][:, 512:1024], hs_a1[:, 0:512])
        nc.vector.tensor_copy(kt_sb[0][:, 512:1024], hs_a1[:, 512:1024])
        nc.vector.tensor_copy(qt_sb[1][:, 0:512], hs_q1[:])
        nc.vector.tensor_copy(kt_sb[1][:, 0:512], hs_k1[:])
        for key in head_start:
            if key[0] != "v":
                proj_done.add(key)
        # V tiles 0-3 right after (xT is fully resident by now), on the
        # 3-deep O ring so drains overlap
        for t_ in range(4):
            emit_v_tile(t_, tag="O")

        # ---- attention (qc-outer, pairs sequential) ----
        for qc in range(QC):
            n_tk = 4 * (qc + 1)
            for p in range(PAIRS):
                for key in seg_needs(qc, p):
                    emit_key(key)  # safety net: force any missing chunk
                ot_ps = [psum_pool.tile([65, 512], f32, tag="O", bufs=3,
                                        name=f"ot_ps{h}") for h in range(2)]
                for tk in range(n_tk):
                    j = tk - 4 * qc  # diagonal sub-position on diag tiles
                    lo = 128 * j if j > 0 else 0
                    s_ps = psum_pool.tile([128, 1024], f32, tag="A",
                                          name="s_ps", bufs=2)
                    for h in range(2):
                        nc.tensor.matmul(
                            s_ps[:, 512 * h + lo:512 * (h + 1)],
                            kt_sb[p][64 * h:64 * (h + 1),
                                     128 * tk:128 * (tk + 1)],
                            qt_sb[p][64 * h:64 * (h + 1),
                                     512 * qc + lo:512 * (qc + 1)],
                            start=True, stop=True,
                            tile_position=(64 * h, 0),
                        )
                    p_sb = p_pool.tile([128, 1024], bf16, tag="p_sb")
                    s3 = s_ps[:].rearrange("p (a q) -> p a q", a=2)
                    p3 = p_sb[:].rearrange("p (a q) -> p a q", a=2)
                    nc.scalar.activation(p3[:, :, lo:512], s3[:, :, lo:512],
                                         Exp, scale=0.125)
                    if j >= 0:
                        # causal staircase: zero entries with q < k in the
                        # 128-wide diagonal block, in place on gpsimd
                        nc.gpsimd.affine_select(
                            out=p3[:, :, lo:lo + 128],
                            in_=p3[:, :, lo:lo + 128],
                            compare_op=mybir.AluOpType.is_ge,
                            fill=0.0,
                            base=0,
                            pattern=[[0, 2], [1, 128]],
                            channel_multiplier=-1,
                        )
                    first, last = (tk == 0), (tk == n_tk - 1)
                    for h in range(2):
                        nc.tensor.matmul(
                            ot_ps[h][:, lo:512],
                            v_all[:, tk, 2 * p + h, :],
                            p_sb[:, 512 * h + lo:512 * (h + 1)],
                            start=first, stop=last,
                        )
                    pop_filler(tk)

                # per head: drain then recip (this order frees the psum
                # ring slot as early as possible for the next segment)
                rfb = [norm_pool.tile([1, 512], bf16, tag=f"rfb{h}",
                                      name=f"rfb{h}") for h in range(2)]
                otraw = [otraw_pool.tile([65, 512], bf16, tag="otraw",
                                         name=f"otraw{h}") for h in range(2)]
                rfs = []
                for h in range(2):
                    rf = norm_pool.tile([65, 512], f32, tag="recipf",
                                        name="recipf", bufs=2)
                    nc.vector.tensor_copy(otraw[h][:], ot_ps[h][:])
                    nc.vector.reciprocal_approx_fast(rf[:], ot_ps[h][:])
                    rfs.append(rf)
                for h in range(2):
                    nc.vector.tensor_copy(rfb[h][:], rfs[h][64:65, :])
                norm_q.append(
                    lambda qc=qc, p=p, r=rfb, o=otraw:
                    emit_norm_tail(qc, p, r, o))

            # after both pairs of qc: queue the output projection of qc
            for t_ in range(4 * qc, 4 * qc + 4):
                for c2 in range(2):
                    out_q.append((t_, c2))

        # tail: remaining norm + out-proj on the freed double-buffered "A"
        # ring so stage copies overlap the matmuls
        while norm_q:
            norm_q.pop(0)()
        while out_q:
            t_, c2 = out_q.pop(0)
            emit_outproj_half(t_, c2, tag="A")

        if dbg is not None:
            dstage_pool = ctx.enter_context(tc.tile_pool(name="dbgstage",
                                                         bufs=2))

            def dump(dst, src_ap, shape):
                st = dstage_pool.tile(shape, f32, tag="dstage", name="dstage")
                nc.vector.tensor_copy(st[:], src_ap)
                nc.sync.dma_start(dst, st[:])
            for p in range(PAIRS):
                dump(dbg["qt"][p], qt_sb[p][:], [128, T])
                dump(dbg["kt"][p], kt_sb[p][:], [128, T])
                dump(dbg["ot"][p], ot_sb[p][:], [128, T])
            dump(dbg["v"][:], v_all[:], [128, TT, 4, 65])
            dump(dbg["wv"][:], wv_sb[:], [128, KC * 256])


def _get_nc():
    if "nc" not in _CACHE:
        _CACHE["nc"] = _build()
    return _CACHE["nc"]


def make_in_maps(x, Wq, Wk, Wv, Wp):
    import ml_dtypes

    bf16 = ml_dtypes.bfloat16
    x = np.asarray(x, dtype=np.float32)
    Wq = np.asarray(Wq, dtype=np.float32).astype(bf16)
    Wk = np.asarray(Wk, dtype=np.float32).astype(bf16)
    Wv = np.asarray(Wv, dtype=np.float32).astype(bf16)
    Wp = np.asarray(Wp, dtype=np.float32).astype(bf16)

    hsel = np.zeros((1, 128), dtype=bf16)
    hsel[0, 0:64] = 1.0

    def sb_weight(wfull):
        # [C, M] -> SBUF layout [128, KC*M]: partition p holds, for each
        # k-chunk, row 128*k+p of the weight
        m = wfull.shape[1]
        return np.ascontiguousarray(
            wfull.reshape(KC, 128, m).transpose(1, 0, 2).reshape(128, -1))

    in_maps = []
    for c in range(N_CORES):
        b = c // 4
        h0 = 4 * (c % 4)  # first of the 4 heads on this core
        hs = list(range(h0, h0 + 4))
        xT = np.ascontiguousarray(x[b].T).astype(bf16)  # [C, T]
        wq = np.stack([sb_weight(np.concatenate(
            [Wq[hs[2 * p]], Wq[hs[2 * p + 1]]], axis=1)) for p in range(PAIRS)])
        wk = np.stack([sb_weight(np.concatenate(
            [Wk[hs[2 * p]], Wk[hs[2 * p + 1]]], axis=1)) for p in range(PAIRS)])
        wv = sb_weight(np.concatenate([Wv[h] for h in hs], axis=1))
        wp = np.stack([Wp[D * h0 + 128 * p:D * h0 + 128 * (p + 1), :]
                       for p in range(PAIRS)])
        in_maps.append({"xT": xT, "wq": wq, "wk": wk, "wv": wv, "wp": wp,
                        "hsel": hsel})
    return in_maps


def kernel(x, Wq, Wk, Wv, Wp):
    from concourse.bass_utils import run_bass_kernel_spmd

    in_maps = make_in_maps(x, Wq, Wk, Wv, Wp)
    nc = _get_nc()
    res = run_bass_kernel_spmd(nc, in_maps, list(range(N_CORES)))

    out = np.zeros((B, T, C), dtype=np.float32)
    for c in range(N_CORES):
        out[c // 4] += res.results[c]["out"]
    return out


# revision 34
# speedup vs baseline: 1.1266x; 1.1092x over previous
"""Bass/Tile TRN2 kernel for nn_MultiHeadAttention (B=2, T=2048, C=1024, H=16, D=64).

Sharding (8 cores): core c -> batch b = c // 4, heads [4*(c%4) .. 4*(c%4)+3]
(tensor-parallel on heads x data-parallel on batch). Each core computes its
4 heads' attention plus its slice of the output projection (rows of Wp for
its heads), producing a partial [T, C]; the host sums the 4 partials per
batch (the "all-reduce" is done host-side since the full output is gathered
host-side anyway).

v4 design (all matmul operands bf16; PSUM accumulate fp32):
  - Host pre-casts x.T and all weights to bf16 AND pre-arranges weights in
    the exact SBUF layout, so every input DMA is contiguous; xT is striped
    over three DMA queues (sync/scalar/vector).
  - k-paced head start: while xT streams in, the PE accumulates Q/K chunk 0
    of both pairs plus V tiles 0-3 chunk-by-chunk, consuming each xT
    k-chunk as it lands.
  - Remaining projections are emitted JUST-IN-TIME as PE filler inside the
    (otherwise ACT-bound) attention loops.  A densely busy PE keeps the HAM
    clock gate at 2.4 GHz (a sparse PE re-throttles to 1.2).
  - Attention per qc chunk of 512 queries, PAIRS SEQUENTIAL: S.T tile =
    K.T.T @ Q.T restricted to unmasked columns, exp on ACT -> bf16 SBUF,
    causal staircase applied in-place by gpsimd affine_select on diagonal
    tiles, PV with fused ones-column rowsum (M=65).  PSUM: s_ps ring
    2x[128,1024] + ot_ps ring 3x[65,512] + filler ring 1x[128,512] = 8
    banks.
  - Normalization decoupled from PE: per head, drain ot_ps -> SBUF bf16
    then reciprocal (order frees the psum ring ASAP); K=1 selector matmuls
    broadcast the recips; bf16 tensor_mul normalizes.  Norm tails are
    popped early (next segment, tile 2); out-projection half-tiles fill
    whatever PE slack remains, spread across the back half of the kernel.
"""

import numpy as np

B, T, C = 2, 2048, 1024
H = 16
D = C // H  # 64
N_CORES = 8
PAIRS = 2  # head-pairs per core
KC = C // 128  # 8 contraction chunks
TT = T // 128  # 16 T tiles
QC = T // 512  # 4 Tq chunks

_CACHE = {}


DEBUG = False


def _build():
    import concourse.mybir as mybir
    import concourse.tile as tile
    from concourse import bacc

    f32 = mybir.dt.float32
    bf16 = mybir.dt.bfloat16

    nc = bacc.Bacc("TRN2", target_bir_lowering=False, debug=False,
                   num_devices=N_CORES)

    # weights host-prearranged: wq/wk [PAIRS, 128, KC*128] (partition-major
    # SBUF layout), wv [128, KC*256], wp [PAIRS, 128, C]
    xT_d = nc.dram_tensor("xT", [C, T], bf16, kind="ExternalInput").ap()
    wq_d = nc.dram_tensor("wq", [PAIRS, 128, KC * 128], bf16,
                          kind="ExternalInput").ap()
    wk_d = nc.dram_tensor("wk", [PAIRS, 128, KC * 128], bf16,
                          kind="ExternalInput").ap()
    wv_d = nc.dram_tensor("wv", [128, KC * 256], bf16,
                          kind="ExternalInput").ap()
    wp_d = nc.dram_tensor("wp", [PAIRS, 128, C], bf16,
                          kind="ExternalInput").ap()
    hsel_d = nc.dram_tensor("hsel", [1, 128], bf16, kind="ExternalInput").ap()
    out_d = nc.dram_tensor("out", [T, C], f32, kind="ExternalOutput").ap()
    dbg = None
    if DEBUG:
        dbg = {
            "qt": nc.dram_tensor("dbg_qt", [PAIRS, 128, T], f32,
                                 kind="ExternalOutput").ap(),
            "kt": nc.dram_tensor("dbg_kt", [PAIRS, 128, T], f32,
                                 kind="ExternalOutput").ap(),
            "v": nc.dram_tensor("dbg_v", [128, TT, 4, 65], f32,
                                kind="ExternalOutput").ap(),
            "ot": nc.dram_tensor("dbg_ot", [PAIRS, 128, T], f32,
                                 kind="ExternalOutput").ap(),
            "wv": nc.dram_tensor("dbg_wv", [128, KC * 256], f32,
                                 kind="ExternalOutput").ap(),
        }

    with tile.TileContext(nc) as tc:
        _emit(nc, tc, tile, mybir, xT_d, wq_d, wk_d, wv_d, wp_d, hsel_d, out_d,
              dbg=dbg)

    nc.compile()
    return nc


def _emit(nc, tc, tile, mybir, xT_d, wq_d, wk_d, wv_d, wp_d, hsel_d, out_d,
          dbg=None):
    from contextlib import ExitStack

    f32 = mybir.dt.float32
    bf16 = mybir.dt.bfloat16
    Exp = mybir.ActivationFunctionType.Exp

    ctx = ExitStack()
    with ctx:
        # ---- pools (everything long-lived: projections span the whole
        # kernel) ----
        qt_pool = ctx.enter_context(tc.tile_pool(name="qt", bufs=1))
        v_pool = ctx.enter_context(tc.tile_pool(name="v", bufs=1))
        ot_pool = ctx.enter_context(tc.tile_pool(name="ot", bufs=1))
        const_pool = ctx.enter_context(tc.tile_pool(name="const", bufs=1))
        wp_pool = ctx.enter_context(tc.tile_pool(name="wp", bufs=1))
        xt_pool = ctx.enter_context(tc.tile_pool(name="xt", bufs=1))
        w_pool = ctx.enter_context(tc.tile_pool(name="w", bufs=1))
        p_pool = ctx.enter_context(tc.tile_pool(name="p", bufs=4))
        otraw_pool = ctx.enter_context(tc.tile_pool(name="otraw", bufs=4))
        norm_pool = ctx.enter_context(tc.tile_pool(name="norm", bufs=2))
        stage_pool = ctx.enter_context(tc.tile_pool(name="stage", bufs=3))
        psum_pool = ctx.enter_context(tc.tile_pool(name="psum", bufs=1,
                                                   space="PSUM"))

        # ---- SBUF tiles ----
        qt_sb = [qt_pool.tile([128, T], bf16, tag=f"qt{p}", name=f"qt{p}")
                 for p in range(PAIRS)]
        kt_sb = [qt_pool.tile([128, T], bf16, tag=f"kt{p}", name=f"kt{p}")
                 for p in range(PAIRS)]
        # v_all[:, tk, hh, 0:64] = V tile for head hh; col 64 = ones (fused
        # rowsum row for the PV matmul)
        v_all = v_pool.tile([128, TT, 4, 65], bf16, tag="vall", name="v_all")
        nc.gpsimd.memset(v_all[:, :, :, 64:65], 1.0)
        ot_sb = [ot_pool.tile([128, T], bf16, tag=f"ot{p}", name=f"ot{p}")
                 for p in range(PAIRS)]
        wq_sb = [w_pool.tile([128, KC * 128], bf16, tag=f"wq{p}",
                             name=f"wq_{p}") for p in range(PAIRS)]
        wk_sb = [w_pool.tile([128, KC * 128], bf16, tag=f"wk{p}",
                             name=f"wk_{p}") for p in range(PAIRS)]
        wv_sb = w_pool.tile([128, KC * 256], bf16, tag="wv", name="wv_sb")
        wp_sb = [wp_pool.tile([128, C], bf16, tag=f"wp{p}", name=f"wpsb{p}")
                 for p in range(PAIRS)]
        xt = [xt_pool.tile([128, T], bf16, tag=f"xt{k}", name=f"xtsb{k}")
              for k in range(KC)]
        hsel_sb = const_pool.tile([1, 128], bf16, tag="hsel", name="hsel_sb")

        # ---- input DMAs: the four Q/K weights + xT (in k order) race ahead
        # on the two HWDGE queues (the k-paced head start consumes each
        # xT chunk as it lands); later-needed weights go via gpsimd SWDGE.
        nc.sync.dma_start(wq_sb[0][:], wq_d[0])
        nc.scalar.dma_start(wk_sb[0][:], wk_d[0])
        nc.sync.dma_start(wq_sb[1][:], wq_d[1])
        nc.scalar.dma_start(wk_sb[1][:], wk_d[1])
        nc.scalar.dma_start(wv_sb[:], wv_d[:])
        qs = [nc.sync, nc.scalar]
        for k in range(KC):
            qs[k % 2].dma_start(xt[k][:], xT_d[128 * k:128 * (k + 1), :])
        nc.scalar.dma_start(wp_sb[0][:], wp_d[0])
        nc.sync.dma_start(wp_sb[1][:], wp_d[1])
        nc.scalar.dma_start(hsel_sb[:], hsel_d[:])

        # warm up the ACT exp table early (hides the 1.3us ACT_TABLE_LOAD)
        warm = const_pool.tile([1, 8], bf16, tag="warm", name="warm")
        nc.gpsimd.memset(warm[:], 0.0)
        nc.scalar.activation(warm[:], warm[:], Exp, scale=0.125)

        # ---- projection chunk emitters (each = 8 accumulating MMs + one
        # DVE drain; popped as PE filler between attention tiles) ----
        proj_done = set()

        def emit_q_chunk(p, c, tag="F"):
            key = ("q", p, c)
            if key in proj_done:
                return
            proj_done.add(key)
            ps = psum_pool.tile([128, 512], f32, tag=tag, name="qch_ps",
                                bufs={"F": 1, "A": 2, "O": 3}[tag])
            for k in range(KC):
                nc.tensor.matmul(
                    ps[:], wq_sb[p][:, 128 * k:128 * (k + 1)],
                    xt[k][:, 512 * c:512 * (c + 1)],
                    start=(k == 0), stop=(k == KC - 1))
            nc.vector.tensor_copy(qt_sb[p][:, 512 * c:512 * (c + 1)], ps[:])

        def emit_k_chunk(p, c, tag="F"):
            key = ("k", p, c)
            if key in proj_done:
                return
            proj_done.add(key)
            ps = psum_pool.tile([128, 512], f32, tag=tag, name="kch_ps",
                                bufs={"F": 1, "A": 2, "O": 3}[tag])
            for k in range(KC):
                nc.tensor.matmul(
                    ps[:], wk_sb[p][:, 128 * k:128 * (k + 1)],
                    xt[k][:, 512 * c:512 * (c + 1)],
                    start=(k == 0), stop=(k == KC - 1))
            nc.vector.tensor_copy(kt_sb[p][:, 512 * c:512 * (c + 1)], ps[:])

        def emit_v_tile(t_, tag="F"):
            key = ("v", t_)
            if key in proj_done:
                return
            proj_done.add(key)
            ps = psum_pool.tile([128, 256], f32, tag=tag, name="v_ps",
                                bufs={"F": 1, "A": 2, "O": 3}[tag])
            for k in range(KC):
                nc.tensor.matmul(
                    ps[:], xt[k][:, 128 * t_:128 * (t_ + 1)],
                    wv_sb[:, 256 * k:256 * (k + 1)],
                    start=(k == 0), stop=(k == KC - 1))
            nc.vector.tensor_copy(
                v_all[:, t_, :, 0:64],
                ps[:].rearrange("p (hh d) -> p hh d", hh=4))

        def emit_outproj_half(t_, c2, tag="F"):
            o_ps = psum_pool.tile([128, 512], f32, tag=tag, name="o_ps",
                                  bufs={"F": 1, "A": 2, "O": 3}[tag])
            for p2 in range(PAIRS):
                nc.tensor.matmul(
                    o_ps[:],
                    ot_sb[p2][:, 128 * t_:128 * (t_ + 1)],
                    wp_sb[p2][:, 512 * c2:512 * (c2 + 1)],
                    start=(p2 == 0), stop=(p2 == PAIRS - 1))
            stage = stage_pool.tile([128, 512], f32, tag="stage", name="stage")
            nc.vector.tensor_copy(stage[:], o_ps[:])
            nc.sync.dma_start(
                out_d[128 * t_:128 * (t_ + 1), 512 * c2:512 * (c2 + 1)],
                stage[:])

        def emit_norm_tail(qc, p, rfb, otraw):
            norm_emitted.add((qc, p))
            # broadcast recip rows across partitions 0-63 via K=1 selector
            # matmuls (base partition 0 so the bf16 tensor_mul has SBUF
            # inputs with matching bases), then normalize into ot_sb
            bc_sb = norm_pool.tile([64, 1024], bf16, tag="bc", name="bc_sb")
            for h in range(2):
                bc_ps = psum_pool.tile([64, 512], f32, tag="F", name="bc_ps",
                                       bufs=1)
                nc.tensor.matmul(bc_ps[:], hsel_sb[:, 0:64], rfb[h][:],
                                 start=True, stop=True)
                nc.vector.tensor_copy(bc_sb[:, 512 * h:512 * (h + 1)],
                                      bc_ps[:])
            for h in range(2):
                nc.vector.tensor_mul(
                    ot_sb[p][64 * h:64 * (h + 1), 512 * qc:512 * (qc + 1)],
                    otraw[h][0:64, :],
                    bc_sb[:, 512 * h:512 * (h + 1)])

        # need-ordered filler queue of projection chunks (what each
        # attention segment requires, minus the pre-emitted head start)
        def seg_needs(qc, p):
            needs = [("v", t_) for t_ in range(4 * qc + 4)]
            needs += [("k", p, c) for c in range(qc + 1)]
            needs += [("q", p, qc)]
            return needs

        SEG_ORDER = [(0, 0), (0, 1), (1, 0), (1, 1),
                     (3, 0), (3, 1), (2, 0), (2, 1)]

        head_start = {("k", 0, 0), ("q", 0, 0), ("k", 0, 1), ("q", 0, 1),
                      ("k", 1, 0), ("q", 1, 0),
                      ("v", 0), ("v", 1), ("v", 2), ("v", 3)}
        proj_queue, seen = [], set(head_start)
        for qc, p in SEG_ORDER:
            for key in seg_needs(qc, p):
                if key not in seen:
                    seen.add(key)
                    proj_queue.append(key)

        def emit_key(key, tag="F"):
            if key[0] == "q":
                emit_q_chunk(key[1], key[2], tag)
            elif key[0] == "k":
                emit_k_chunk(key[1], key[2], tag)
            else:
                emit_v_tile(key[1], tag)

        norm_q = []  # norm-tail closures: popped at tile 2 of the next seg
        out_q = []   # out-proj half-tiles (t_, c2): fill remaining PE slack
        norm_emitted = set()  # (qc, p) pairs whose norm tail has been emitted

        def pop_outs(limit):
            # out-proj of qc is only valid once BOTH pairs' norm tails are
            # emitted (emission order, not just semaphores: an earlier read
            # would see stale SBUF)
            done = 0
            i = 0
            while i < len(out_q) and done < limit:
                t_, c2 = out_q[i]
                qc_t = t_ // 4
                if (qc_t, 0) in norm_emitted and (qc_t, 1) in norm_emitted:
                    out_q.pop(i)
                    emit_outproj_half(t_, c2)
                    done += 1
                else:
                    i += 1
            return done

        def pop_filler(tk):
            if tk >= 1 and norm_q:
                norm_q.pop(0)()
                return
            while proj_queue and proj_queue[0] in proj_done:
                proj_queue.pop(0)
            if proj_queue:
                emit_key(proj_queue.pop(0))
                return
            pop_outs(2)

        # ---- k-paced head start: consume each xT chunk as it lands,
        # accumulating Q/K chunk 0 of both pairs + Q/K chunk 1 of pair 0
        # (each accumulation group gets its own PSUM bank).
        # PSUM: 2 tag-A slots [128,1024] (2 banks each) + 2 tag-O slots.
        hs_a0 = psum_pool.tile([128, 1024], f32, tag="A", bufs=2,
                               name="hs_a0")
        hs_a1 = psum_pool.tile([128, 1024], f32, tag="A", bufs=2,
                               name="hs_a1")
        hs_q1 = psum_pool.tile([128, 512], f32, tag="O", bufs=3, name="hs_q1")
        hs_k1 = psum_pool.tile([128, 512], f32, tag="O", bufs=3, name="hs_k1")
        hs_mms = [
            (hs_a0[:, 0:512], wq_sb[0], 0),    # Q p0 c0
            (hs_a0[:, 512:1024], wk_sb[0], 0),  # K p0 c0
            (hs_a1[:, 0:512], wq_sb[0], 1),    # Q p0 c1
            (hs_a1[:, 512:1024], wk_sb[0], 1),  # K p0 c1
            (hs_q1[:], wq_sb[1], 0),           # Q p1 c0
            (hs_k1[:], wk_sb[1], 0),           # K p1 c0
        ]
        for k in range(KC):
            st, sp = (k == 0), (k == KC - 1)
            for (dst, w_, c) in hs_mms:
                nc.tensor.matmul(dst, w_[:, 128 * k:128 * (k + 1)],
                                 xt[k][:, 512 * c:512 * (c + 1)],
                                 start=st, stop=sp, skip_group_check=True)
        nc.vector.tensor_copy(qt_sb[0][:, 0:512], hs_a0[:, 0:512])
        nc.vector.tensor_copy(kt_sb[0][:, 0:512], hs_a0[:, 512:1024])
        nc.vector.tensor_copy(qt_sb[0][:, 512:1024], hs_a1[:, 0:512])
        nc.vector.tensor_copy(kt_sb[0][:, 512:1024], hs_a1[:, 512:1024])
        nc.vector.tensor_copy(qt_sb[1][:, 0:512], hs_q1[:])
        nc.vector.tensor_copy(kt_sb[1][:, 0:512], hs_k1[:])
        for key in head_start:
            if key[0] != "v":
                proj_done.add(key)
        # V tiles 0-3 right after (xT is fully resident by now), on the
        # 3-deep O ring so drains overlap
        for t_ in range(4):
            emit_v_tile(t_, tag="O")

        # ---- attention (segments per SEG_ORDER, pairs sequential) ----
        for qc, p in SEG_ORDER:
            n_tk = 4 * (qc + 1)
            if True:
                for key in seg_needs(qc, p):
                    emit_key(key)  # safety net: force any missing chunk
                ot_ps = [psum_pool.tile([65, 512], f32, tag="O", bufs=3,
                                        name=f"ot_ps{h}") for h in range(2)]
                for tk in range(n_tk):
                    j = tk - 4 * qc  # diagonal sub-position on diag tiles
                    lo = 128 * j if j > 0 else 0
                    s_ps = psum_pool.tile([128, 1024], f32, tag="A",
                                          name="s_ps", bufs=2)
                    for h in range(2):
                        nc.tensor.matmul(
                            s_ps[:, 512 * h + lo:512 * (h + 1)],
                            kt_sb[p][64 * h:64 * (h + 1),
                                     128 * tk:128 * (tk + 1)],
                            qt_sb[p][64 * h:64 * (h + 1),
                                     512 * qc + lo:512 * (qc + 1)],
                            start=True, stop=True,
                            tile_position=(64 * h, 0),
                        )
                    p_sb = p_pool.tile([128, 1024], bf16, tag="p_sb")
                    s3 = s_ps[:].rearrange("p (a q) -> p a q", a=2)
                    p3 = p_sb[:].rearrange("p (a q) -> p a q", a=2)
                    nc.scalar.activation(p3[:, :, lo:512], s3[:, :, lo:512],
                                         Exp, scale=0.125)
                    if j >= 0:
                        # causal staircase: zero entries with q < k in the
                        # 128-wide diagonal block, in place on gpsimd
                        nc.gpsimd.affine_select(
                            out=p3[:, :, lo:lo + 128],
                            in_=p3[:, :, lo:lo + 128],
                            compare_op=mybir.AluOpType.is_ge,
                            fill=0.0,
                            base=0,
                            pattern=[[0, 2], [1, 128]],
                            channel_multiplier=-1,
                        )
                    first, last = (tk == 0), (tk == n_tk - 1)
                    for h in range(2):
                        nc.tensor.matmul(
                            ot_ps[h][:, lo:512],
                            v_all[:, tk, 2 * p + h, :],
                            p_sb[:, 512 * h + lo:512 * (h + 1)],
                            start=first, stop=last,
                        )
                    pop_filler(tk)

                # per head: drain then recip (this order frees the psum
                # ring slot as early as possible for the next segment)
                rfb = [norm_pool.tile([1, 512], bf16, tag=f"rfb{h}",
                                      name=f"rfb{h}") for h in range(2)]
                otraw = [otraw_pool.tile([65, 512], bf16, tag="otraw",
                                         name=f"otraw{h}") for h in range(2)]
                rfs = []
                for h in range(2):
                    rf = norm_pool.tile([65, 512], f32, tag="recipf",
                                        name="recipf", bufs=2)
                    nc.vector.tensor_copy(otraw[h][:], ot_ps[h][:])
                    nc.vector.reciprocal_approx_fast(rf[:], ot_ps[h][:])
                    rfs.append(rf)
                for h in range(2):
                    nc.vector.tensor_copy(rfb[h][:], rfs[h][64:65, :])
                norm_q.append(
                    lambda qc=qc, p=p, r=rfb, o=otraw:
                    emit_norm_tail(qc, p, r, o))

            # after both pairs of qc: queue the output projection of qc
            if p == 1:
                for t_ in range(4 * qc, 4 * qc + 4):
                    for c2 in range(2):
                        out_q.append((t_, c2))

        # tail: remaining norm + out-proj on the freed double-buffered "A"
        # ring so stage copies overlap the matmuls
        while norm_q:
            norm_q.pop(0)()
        while out_q:
            t_, c2 = out_q.pop(0)
            emit_outproj_half(t_, c2, tag="A")

        if dbg is not None:
            dstage_pool = ctx.enter_context(tc.tile_pool(name="dbgstage",
                                                         bufs=2))

            def dump(dst, src_ap, shape):
                st = dstage_pool.tile(shape, f32, tag="dstage", name="dstage")
                nc.vector.tensor_copy(st[:], src_ap)
                nc.sync.dma_start(dst, st[:])
            for p in range(PAIRS):
                dump(dbg["qt"][p], qt_sb[p][:], [128, T])
                dump(dbg["kt"][p], kt_sb[p][:], [128, T])
                dump(dbg["ot"][p], ot_sb[p][:], [128, T])
            dump(dbg["v"][:], v_all[:], [128, TT, 4, 65])
            dump(dbg["wv"][:], wv_sb[:], [128, KC * 256])


def _get_nc():
    if "nc" not in _CACHE:
        _CACHE["nc"] = _build()
    return _CACHE["nc"]


def make_in_maps(x, Wq, Wk, Wv, Wp):
    import ml_dtypes

    bf16 = ml_dtypes.bfloat16
    x = np.asarray(x, dtype=np.float32)
    Wq = np.asarray(Wq, dtype=np.float32).astype(bf16)
    Wk = np.asarray(Wk, dtype=np.float32).astype(bf16)
    Wv = np.asarray(Wv, dtype=np.float32).astype(bf16)
    Wp = np.asarray(Wp, dtype=np.float32).astype(bf16)

    hsel = np.zeros((1, 128), dtype=bf16)
    hsel[0, 0:64] = 1.0

    def sb_weight(wfull):
        # [C, M] -> SBUF layout [128, KC*M]: partition p holds, for each
        # k-chunk, row 128*k+p of the weight
        m = wfull.shape[1]
        return np.ascontiguousarray(
            wfull.reshape(KC, 128, m).transpose(1, 0, 2).reshape(128, -1))

    in_maps = []
    for c in range(N_CORES):
        b = c // 4
        h0 = 4 * (c % 4)  # first of the 4 heads on this core
        hs = list(range(h0, h0 + 4))
        xT = np.ascontiguousarray(x[b].T).astype(bf16)  # [C, T]
        wq = np.stack([sb_weight(np.concatenate(
            [Wq[hs[2 * p]], Wq[hs[2 * p + 1]]], axis=1)) for p in range(PAIRS)])
        wk = np.stack([sb_weight(np.concatenate(
            [Wk[hs[2 * p]], Wk[hs[2 * p + 1]]], axis=1)) for p in range(PAIRS)])
        wv = sb_weight(np.concatenate([Wv[h] for h in hs], axis=1))
        wp = np.stack([Wp[D * h0 + 128 * p:D * h0 + 128 * (p + 1), :]
                       for p in range(PAIRS)])
        in_maps.append({"xT": xT, "wq": wq, "wk": wk, "wv": wv, "wp": wp,
                        "hsel": hsel})
    return in_maps


def kernel(x, Wq, Wk, Wv, Wp):
    from concourse.bass_utils import run_bass_kernel_spmd

    in_maps = make_in_maps(x, Wq, Wk, Wv, Wp)
    nc = _get_nc()
    res = run_bass_kernel_spmd(nc, in_maps, list(range(N_CORES)))

    out = np.zeros((B, T, C), dtype=np.float32)
    for c in range(N_CORES):
        out[c // 4] += res.results[c]["out"]
    return out
